# revision 1
# baseline (speedup 1.0000x reference)
"""Trainium2 Bass kernel for the correlation-map embedding module.

Math (per (b, nf) pair):
  f1d = bilinear_down28(feature_i[b, nf])                  # [C, 28, 28]
  f2sel[c, k] = bilinear sample of feature_j[b, nf] at the K knn grid points
  corr[k, :, :] = relu(sum_c f2sel[c, k] * f1d[c, :, :])   # [K, 28, 28]
  out[k] = corr[k] / sum_hw(exp(corr[k])) * 10

Key restructurings vs the reference:
  - only the K=128 selected query positions of f2 are ever computed (4-tap
    weighted gather: ap_gather on GPSIMD + weighting on DVE + the tap
    reduction folded into 4 accumulating matmuls), not the full 784 grid;
  - bilinear taps are exactly (2k, 2k+1) per output index, so the f1
    downsample is a single fused 4-tap weighted sum at 28x28 resolution:
    7 DVE ops on 784-elem tiles with precomputed product-weight planes;
  - the channel contraction runs on the tensor engine in float32r
    (full PE rate, ~1e-3 relative accuracy wrt fp32);
  - epilogue scaling rides the ScalarE activations: relu(corr)*10 via
    activation scale, exp(relu(corr)) via Exp with scale=0.1, final
    normalize via Copy with per-partition scale = 1/denom.

Sharding: pure data parallel — batch dim (16) split across 8 cores, 2 each.
"""

import numpy as np

# hardcoded problem shapes (grading calls kernel(**inputs) standalone)
B, NF, C, H, W = 16, 3, 128, 56, 56
G = 28
K = 128
NCORES = 8
BPC = B // NCORES  # 2
P = 128

_CACHE = {}


def _axis_coords(n_in):
    # float32 arithmetic to match the jax reference bit-for-bit
    src = np.arange(G, dtype=np.float32) * np.float32((n_in - 1) / (G - 1))
    i0 = np.clip(np.floor(src).astype(np.int32), 0, n_in - 2)
    w = (src - i0.astype(np.float32)).astype(np.float32)
    return i0, w


def _host_consts(knn_inds):
    i0h, wh = _axis_coords(H)
    i0w, ww = _axis_coords(W)
    # the even/odd strided-AP downsample assumes taps are (2k, 2k+1)
    assert np.array_equal(i0h, 2 * np.arange(G)) and np.array_equal(i0w, 2 * np.arange(G))

    # fused 4-tap downsample product-weight planes, each [28*28]
    # tap order (u, t): u = H-axis tap, t = W-axis tap
    ah, bh = (1.0 - wh), wh
    aw, bw = (1.0 - ww), ww
    w4 = np.stack(
        [
            np.outer(ah, aw).reshape(-1),
            np.outer(ah, bw).reshape(-1),
            np.outer(bh, aw).reshape(-1),
            np.outer(bh, bw).reshape(-1),
        ]
    ).astype(np.float32)  # [4, 784]

    # gather indices/weights for the 4 bilinear taps of each knn point
    knn = np.asarray(knn_inds).astype(np.int64)  # [NF, K, 2]
    gidx = np.zeros((NF, P, 16), dtype=np.int16)
    gidx2 = np.zeros((NF, P, 32), dtype=np.int16)
    gwts = np.zeros((NF, 4 * K), dtype=np.float32)
    for nf in range(NF):
        h2 = knn[nf, :, 1]
        w2 = knn[nf, :, 0]
        r0 = i0h[h2]
        c0 = i0w[w2]
        # d=2 gather: each index fetches the horizontally-contiguous tap pair
        # (r_u*W + c0, +1); index is in units of 2 elements (c0 even).
        # j = k*2 + u ordering: gathered tile is [P, K, 2, 2] = [P, K, 4]
        pos = np.stack(
            [(r0 * W + c0) // 2, ((r0 + 1) * W + c0) // 2], axis=1
        ).reshape(-1)  # [256]
        wt = np.stack(
            [ah[h2] * aw[w2], ah[h2] * bw[w2], bh[h2] * aw[w2], bh[h2] * bw[w2]],
            axis=1,
        ).reshape(-1)
        gwts[nf] = wt.astype(np.float32)
        # ap_gather index layout: gathered index j comes from partition j%16,
        # slot j//16 of its 16-partition group; replicate across the 8 groups
        wrapped = pos.reshape(16, 16).T.astype(np.int16)  # [16, 16]
        gidx[nf] = np.tile(wrapped, (8, 1))
        # merged variant: one gather per nf covering both batches stacked in
        # one [P, 2*H*W] tile; j = b*256 + k*2 + u, b offset in d=2 units
        pos2 = np.concatenate([pos, pos + H * W // 2])  # [512]
        wrapped2 = pos2.reshape(32, 16).T.astype(np.int16)  # [16, 32]
        gidx2[nf] = np.tile(wrapped2, (8, 1))
    return w4, gidx, gidx2, gwts


def _build_bass(repeat=1, mode="full"):
    """mode: "full" = real kernel; "dma" = only the DMA traffic (roofline probe).
    repeat: clone the whole per-pair pipeline R times (idempotent) so HW time
    can be measured by differencing two R values."""
    import concourse.bacc as bacc
    import concourse.tile as tile
    from concourse import mybir

    f32 = mybir.dt.float32
    f32r = mybir.dt.float32r
    i16 = mybir.dt.int16
    AF = mybir.ActivationFunctionType
    ALU = mybir.AluOpType

    nc = bacc.Bacc()
    fi = nc.dram_tensor("fi", [BPC, NF, C, H, W], f32, kind="ExternalInput")
    fj = nc.dram_tensor("fj", [BPC, NF, C, H, W], f32, kind="ExternalInput")
    w4_d = nc.dram_tensor("w4", [1, 4 * G * G + P], f32r, kind="ExternalInput")
    gidx_d = nc.dram_tensor("gidx", [NF, P, 16], i16, kind="ExternalInput")
    gidx2_d = nc.dram_tensor("gidx2", [NF, P, 32], i16, kind="ExternalInput")
    gw_d = nc.dram_tensor("gw", [1, NF * 4 * K], f32r, kind="ExternalInput")
    out_d = nc.dram_tensor("out", [BPC, NF, K, G, G], f32, kind="ExternalOutput")

    with tile.TileContext(nc) as tc:
        with (
            tc.tile_pool(name="consts", bufs=1) as consts,
            tc.tile_pool(name="feat2x", bufs=2) as feat2x,
            tc.tile_pool(name="feat1", bufs=2) as feat1,
            tc.tile_pool(name="work", bufs=2) as work,
            tc.tile_pool(name="psum", bufs=3, space="PSUM") as pspool,
            tc.tile_pool(name="bcpsum", bufs=2, space="PSUM") as bcpool,
            tc.tile_pool(name="outp", bufs=3) as outp,
        ):
            # constants: load single-partition rows from HBM (tiny), then
            # replicate across partitions with ones-vector matmuls on the idle
            # PE — avoids 2.3MB of broadcast DMA traffic on the memory-bound
            # critical path. float32r rounding of the weights (~1e-3) is in
            # the same class as the correlation matmul's own rounding.
            w4row = consts.tile([1, 4 * G * G + P], f32r, tag="w4row")
            nc.scalar.dma_start(out=w4row, in_=w4_d[:, :])
            gwrow = consts.tile([1, NF * 4 * K], f32r, tag="gwrow")
            nc.scalar.dma_start(out=gwrow, in_=gw_d[:, :])
            # trailing P entries of the w4 input are 1.0: the ones row for
            # the PE partition-broadcast matmuls
            ones = w4row[:, 4 * G * G : 4 * G * G + P]

            def pe_broadcast(row_ap, n):
                """[1, n] -> [P, n] via PE: out = ones.T @ row."""
                dst = consts.tile([P, n], f32, tag=f"bc{len(bc_tiles)}")
                done = 0
                while done < n:
                    chunk = min(512, n - done)
                    bps = bcpool.tile([P, 512], f32, tag="bps")
                    nc.tensor.matmul(
                        bps[:, :chunk],
                        lhsT=ones,
                        rhs=row_ap[:, done : done + chunk],
                        start=True,
                        stop=True,
                    )
                    nc.scalar.copy(dst[:, done : done + chunk], bps[:, :chunk])
                    done += chunk
                bc_tiles.append(dst)
                return dst

            bc_tiles = []
            w4_t = [
                pe_broadcast(w4row[:, u * G * G : (u + 1) * G * G], G * G)
                for u in range(4)
            ]
            gw_t = [
                pe_broadcast(gwrow[:, nf * 4 * K : (nf + 1) * 4 * K], 4 * K)
                for nf in range(NF)
            ]
            gidx_t = []
            gidx2_t = []
            for nf in range(NF):
                it = consts.tile([P, 16], i16, tag=f"gidx{nf}")
                nc.scalar.dma_start(out=it, in_=gidx_d[nf])
                gidx_t.append(it)
                it2 = consts.tile([P, 32], i16, tag=f"gidx2_{nf}")
                nc.scalar.dma_start(out=it2, in_=gidx2_d[nf])
                gidx2_t.append(it2)

            import contextlib

            loop_ctx = (
                tc.For_i(0, repeat, 1) if repeat > 1 else contextlib.nullcontext()
            )
            HH = H // 2  # 28 rows per half
            GH = G // 2  # 14 output rows per half
            merged = mode in ("full", "dma")
            with loop_ctx:
              for nf in range(NF):
                # f1 halves for both batches FIRST: the downsample (the bulk
                # of DVE work) streams while f2 is still loading, so no DVE
                # backlog trails the last DMA
                f1hs = {}
                for b in range(BPC):
                    f1hs[b] = []
                    for h in range(2):
                        t = feat1.tile([P, HH, W], f32, tag=f"f1h{b}_{h}")
                        nc.sync.dma_start(
                            out=t, in_=fi[b, nf, :, h * HH : (h + 1) * HH, :]
                        )
                        f1hs[b].append(t)
                if merged and mode != "dma":
                    # both batches' f2 stacked in one tile; single gather per
                    # nf amortizes the ap_gather fixed cost
                    f2x = feat2x.tile([P, BPC, H * W], f32, tag="f2x")
                    for b in range(BPC):
                        nc.sync.dma_start(
                            out=f2x[:, b, :],
                            in_=fj[b, nf].rearrange("p h w -> p (h w)"),
                        )
                    g2 = work.tile([P, BPC, K, 4], f32, tag="g2")
                    nc.gpsimd.ap_gather(
                        g2.rearrange("p b k t -> p (b k t)"),
                        f2x.rearrange("p b q -> p (b q)"),
                        gidx2_t[nf],
                        channels=P,
                        num_elems=BPC * H * W // 2,
                        d=2,
                        num_idxs=BPC * 2 * K,
                    )
                for b in range(BPC):
                    f1h = f1hs[b]
                    if not merged:
                        f2 = feat2x.tile([P, H, W], f32, tag="f2")
                        nc.sync.dma_start(out=f2, in_=fj[b, nf])

                    if mode == "dma":
                        # inputs: fj loaded once per (b, nf) like full
                        f2 = feat2x.tile([P, H, W], f32, tag="f2")
                        nc.sync.dma_start(out=f2, in_=fj[b, nf])
                        o = outp.tile([P, G * G], f32, tag="o")
                        nc.vector.memset(o, 0.0)
                        nc.scalar.dma_start(
                            out=out_d[b, nf].rearrange("k g1 g2 -> k (g1 g2)"), in_=o
                        )
                        continue

                    # f2 at the K selected grid points: gather the 4 bilinear
                    # taps (GPSIMD), apply tap weights (DVE)
                    if merged:
                        g = g2[:, b]
                    else:
                        g = work.tile([P, K, 4], f32, tag="g")
                        if mode == "nogather":
                            nc.vector.tensor_copy(
                                g.rearrange("p k t -> p (k t)"),
                                f2.rearrange("p h w -> p (h w)")[:, : 4 * K],
                            )
                        else:
                            nc.gpsimd.ap_gather(
                                g.rearrange("p k t -> p (k t)"),
                                f2.rearrange("p h w -> p (h w)"),
                                gidx_t[nf],
                                channels=P,
                                num_elems=H * W // 2,
                                d=2,
                                num_idxs=2 * K,
                            )
                    gg = work.tile([P, K, 4], f32r, tag="gg")
                    nc.vector.tensor_mul(
                        gg.rearrange("p k t -> p (k t)"),
                        g.rearrange("p k t -> p (k t)"),
                        gw_t[nf],
                    )

                    # per half: fused 4-tap downsample (DVE) + 4 accumulating
                    # matmuls; corr[k, q] = sum_c sum_t gg[c,k,t] * f1d[c,q]
                    ps = pspool.tile([P, 2, 512], f32, tag="ps")
                    for h in range(2):
                        f1v = f1h[h].rearrange(
                            "p (h uu) (w tt) -> p h uu w tt", uu=2, tt=2
                        )
                        m = []
                        for u in range(2):
                            for t in range(2):
                                mt = work.tile([P, GH, G], f32, tag=f"m{u}{t}")
                                nc.vector.tensor_mul(
                                    mt,
                                    f1v[:, :, u, :, t],
                                    w4_t[2 * u + t].rearrange(
                                        "p (h g) -> p h g", g=G
                                    )[:, h * GH : (h + 1) * GH, :],
                                )
                                m.append(mt)
                        a0 = work.tile([P, GH, G], f32, tag="a0")
                        nc.vector.tensor_add(a0, m[0], m[1])
                        a1 = work.tile([P, GH, G], f32, tag="a1")
                        nc.vector.tensor_add(a1, m[2], m[3])
                        f1d = work.tile([P, GH, G], f32r, tag="f1d")
                        nc.vector.tensor_add(f1d, a0, a1)

                        rhs = f1d.rearrange("p h g -> p (h g)")  # [P, 392]
                        for t in range(4):
                            nc.tensor.matmul(
                                ps[:, h, : GH * G],
                                lhsT=gg[:, :, t],
                                rhs=rhs,
                                start=(t == 0),
                                stop=(t == 3),
                            )

                    # epilogue on ScalarE: r = 10*relu(corr); s = sum(exp(r/10));
                    # out = r * (1/s)
                    r = outp.tile([P, 2, GH * G], f32, tag="r")
                    nc.scalar.activation(r, ps[:, :, : GH * G], AF.Relu, scale=10.0)
                    rf = r.rearrange("p h q -> p (h q)")  # [P, 784] contiguous
                    e = work.tile([P, G * G], f32, tag="e")
                    s = work.tile([P, 1], f32, tag="s")
                    nc.scalar.activation(e, rf, AF.Exp, scale=0.1, accum_out=s)
                    rec = work.tile([P, 1], f32, tag="rec")
                    nc.vector.reciprocal(rec, s)
                    o = outp.tile([P, G * G], f32, tag="o")
                    # final normalize on DVE (tensor_scalar runs in 2x mode)
                    nc.vector.tensor_scalar(
                        o, rf, rec, None, op0=ALU.mult
                    )
                    # issue the store from ScalarE (mostly idle): keeps the
                    # SP/sync stream free to prefetch later pairs instead of
                    # stalling on this pair's compute chain
                    nc.scalar.dma_start(
                        out=out_d[b, nf].rearrange("k g1 g2 -> k (g1 g2)"), in_=o
                    )
    return nc


def _get_bass():
    if "nc" not in _CACHE:
        nc = _build_bass()
        # run the Bacc passes (reg alloc, library-load insertion) before the
        # PJRT path serializes the module
        if not nc.is_finalized():
            nc.finalize()
        _CACHE["nc"] = nc
    return _CACHE["nc"]


def kernel(feature_i, feature_j, mask, optical_flow, knn_inds):
    from concourse import bass_utils

    nc = _get_bass()
    w4, gidx, gidx2, gwts = _host_consts(knn_inds)

    fi = np.ascontiguousarray(np.asarray(feature_i, dtype=np.float32))
    fj = np.ascontiguousarray(np.asarray(feature_j, dtype=np.float32))

    in_maps = []
    for core in range(NCORES):
        lo = core * BPC
        in_maps.append(
            {
                "fi": fi[lo : lo + BPC],
                "fj": fj[lo : lo + BPC],
                "w4": np.concatenate([w4.reshape(-1), np.ones(P, np.float32)])[None, :],
                "gidx": gidx,
                "gidx2": gidx2,
                "gw": gwts.reshape(1, -1),
            }
        )

    res = bass_utils.run_bass_kernel_spmd(nc, in_maps, core_ids=list(range(NCORES)))
    out = np.concatenate([res.results[c]["out"] for c in range(NCORES)], axis=0)
    return out.astype(np.float32)



# revision 2
# speedup vs baseline: 1.1030x; 1.1030x over previous
"""Trainium2 Bass kernel for the correlation-map embedding module (v2).

Math (per (b, nf) pair):
  f1d = bilinear_down28(feature_i[b, nf])                  # [C, 28, 28]
  f2sel[c, k] = bilinear sample of feature_j[b, nf] at the K knn grid points
  corr[k, :, :] = relu(sum_c f2sel[c, k] * f1d[c, :, :])   # [K, 28, 28]
  out[k] = corr[k] / sum_hw(exp(corr[k])) * 10

v2 restructurings (vs the v1 "4 accumulating matmuls per tap" design):
  - the f2 tap reduction is pre-summed on DVE (2 strided adds, 384 elems)
    giving a SINGLE stationary lhsT f2sel[c,k] per pair instead of 4;
  - the f1 downsample's 3 tap adds are folded into PSUM accumulation:
    4 accumulating matmuls per PSUM bank against the 4 weighted tap
    planes (same lhsT), so DVE only does the 4 w4-muls (full 28x28 grid,
    one op each) instead of 7 ops per half;
  - epilogue normalize moved from DVE to ScalarE (Copy with per-partition
    scale = 1/denom), leaving DVE with just the reciprocal.
  Net: DVE work per pair drops ~9.7us -> ~5.3us, PE work drops to 8 small
  matmuls; DMA (21.7MB @ ~358GB/s per core) becomes the only roofline.

Sharding: pure data parallel - batch dim (16) split across 8 cores, 2 each.
"""

import numpy as np

# hardcoded problem shapes (grading calls kernel(**inputs) standalone)
B, NF, C, H, W = 16, 3, 128, 56, 56
G = 28
K = 128
NCORES = 8
BPC = B // NCORES  # 2
P = 128
QH = G * G // 2  # 392 psum columns per bank

_CACHE = {}


def _axis_coords(n_in):
    # float32 arithmetic to match the jax reference bit-for-bit
    src = np.arange(G, dtype=np.float32) * np.float32((n_in - 1) / (G - 1))
    i0 = np.clip(np.floor(src).astype(np.int32), 0, n_in - 2)
    w = (src - i0.astype(np.float32)).astype(np.float32)
    return i0, w


def _host_consts(knn_inds):
    i0h, wh = _axis_coords(H)
    i0w, ww = _axis_coords(W)
    # the even/odd strided-AP downsample assumes taps are (2k, 2k+1)
    assert np.array_equal(i0h, 2 * np.arange(G)) and np.array_equal(i0w, 2 * np.arange(G))

    # fused 4-tap downsample product-weight planes, each [28*28]
    # tap order (u, t): u = H-axis tap, t = W-axis tap
    ah, bh = (1.0 - wh), wh
    aw, bw = (1.0 - ww), ww
    w4 = np.stack(
        [
            np.outer(ah, aw).reshape(-1),
            np.outer(ah, bw).reshape(-1),
            np.outer(bh, aw).reshape(-1),
            np.outer(bh, bw).reshape(-1),
        ]
    ).astype(np.float32)  # [4, 784]

    # gather indices/weights for the 4 bilinear taps of each knn point
    knn = np.asarray(knn_inds).astype(np.int64)  # [NF, K, 2]
    gidx2 = np.zeros((NF, P, 32), dtype=np.int16)
    gwts = np.zeros((NF, 4 * K), dtype=np.float32)
    for nf in range(NF):
        h2 = knn[nf, :, 1]
        w2 = knn[nf, :, 0]
        r0 = i0h[h2]
        c0 = i0w[w2]
        # d=2 gather: each index fetches the horizontally-contiguous tap pair
        # (r_u*W + c0, +1); index is in units of 2 elements (c0 even).
        # j = k*2 + u ordering: gathered tile is [P, K, 2, 2] = [P, K, 4]
        pos = np.stack(
            [(r0 * W + c0) // 2, ((r0 + 1) * W + c0) // 2], axis=1
        ).reshape(-1)  # [256]
        wt = np.stack(
            [ah[h2] * aw[w2], ah[h2] * bw[w2], bh[h2] * aw[w2], bh[h2] * bw[w2]],
            axis=1,
        ).reshape(-1)
        gwts[nf] = wt.astype(np.float32)
        # merged gather: one gather per nf covering both batches stacked in
        # one [P, 2*H*W] tile; j = b*256 + k*2 + u, b offset in d=2 units.
        # ap_gather index layout: gathered index j comes from partition j%16,
        # slot j//16 of its 16-partition group; replicate across the 8 groups
        pos2 = np.concatenate([pos, pos + H * W // 2])  # [512]
        wrapped2 = pos2.reshape(32, 16).T.astype(np.int16)  # [16, 32]
        gidx2[nf] = np.tile(wrapped2, (8, 1))
    return w4, gidx2, gwts


def _build_bass(mode="full"):
    """mode: "full" = real kernel; "dma" = only the DMA traffic (roofline probe)."""
    import concourse.bacc as bacc
    import concourse.tile as tile
    from concourse import mybir

    f32 = mybir.dt.float32
    f32r = mybir.dt.float32r
    i16 = mybir.dt.int16
    AF = mybir.ActivationFunctionType

    nc = bacc.Bacc()
    fi = nc.dram_tensor("fi", [BPC, NF, C, H, W], f32, kind="ExternalInput")
    fj = nc.dram_tensor("fj", [BPC, NF, C, H, W], f32, kind="ExternalInput")
    w4_d = nc.dram_tensor("w4", [1, 4 * G * G + P], f32r, kind="ExternalInput")
    gidx2_d = nc.dram_tensor("gidx2", [NF, P, 32], i16, kind="ExternalInput")
    gw_d = nc.dram_tensor("gw", [1, NF * 4 * K], f32r, kind="ExternalInput")
    out_d = nc.dram_tensor("out", [BPC, NF, K, G, G], f32, kind="ExternalOutput")

    with tile.TileContext(nc) as tc:
        with (
            tc.tile_pool(name="consts", bufs=1) as consts,
            tc.tile_pool(name="feat2x", bufs=2) as feat2x,
            tc.tile_pool(name="feat1", bufs=2) as feat1,
            tc.tile_pool(name="work", bufs=2) as work,
            tc.tile_pool(name="psum", bufs=2, space="PSUM") as pspool,
            tc.tile_pool(name="bcpsum", bufs=2, space="PSUM") as bcpool,
            tc.tile_pool(name="outp", bufs=3) as outp,
        ):
            # constants: load single-partition rows from HBM (tiny), then
            # replicate across partitions with ones-vector matmuls on the idle
            # PE - avoids 2.3MB of broadcast DMA traffic on the memory-bound
            # critical path. float32r rounding of the weights (~1e-3) is in
            # the same class as the correlation matmul's own rounding.
            w4row = consts.tile([1, 4 * G * G + P], f32r, tag="w4row")
            nc.scalar.dma_start(out=w4row, in_=w4_d[:, :])
            gwrow = consts.tile([1, NF * 4 * K], f32r, tag="gwrow")
            nc.scalar.dma_start(out=gwrow, in_=gw_d[:, :])
            # trailing P entries of the w4 input are 1.0: the ones row for
            # the PE partition-broadcast matmuls
            ones = w4row[:, 4 * G * G : 4 * G * G + P]

            bc_tiles = []

            def pe_broadcast(row_ap, n):
                """[1, n] -> [P, n] via PE: out = ones.T @ row."""
                dst = consts.tile([P, n], f32, tag=f"bc{len(bc_tiles)}")
                done = 0
                while done < n:
                    chunk = min(512, n - done)
                    bps = bcpool.tile([P, 512], f32, tag="bps")
                    nc.tensor.matmul(
                        bps[:, :chunk],
                        lhsT=ones,
                        rhs=row_ap[:, done : done + chunk],
                        start=True,
                        stop=True,
                    )
                    nc.scalar.copy(dst[:, done : done + chunk], bps[:, :chunk])
                    done += chunk
                bc_tiles.append(dst)
                return dst

            w4_t = [
                pe_broadcast(w4row[:, u * G * G : (u + 1) * G * G], G * G)
                for u in range(4)
            ]
            gw_t = [
                pe_broadcast(gwrow[:, nf * 4 * K : (nf + 1) * 4 * K], 4 * K)
                for nf in range(NF)
            ]
            gidx2_t = []
            for nf in range(NF):
                it2 = consts.tile([P, 32], i16, tag=f"gidx2_{nf}")
                nc.scalar.dma_start(out=it2, in_=gidx2_d[nf])
                gidx2_t.append(it2)

            for nf in range(NF):
                # all 4 big loads first so the SDMA fabric stays saturated
                f1t = []
                for b in range(BPC):
                    t = feat1.tile([P, H * W], f32, tag=f"f1_{b}")
                    nc.sync.dma_start(
                        out=t, in_=fi[b, nf].rearrange("p h w -> p (h w)")
                    )
                    f1t.append(t)
                f2x = feat2x.tile([P, BPC, H * W], f32, tag="f2x")
                for b in range(BPC):
                    nc.sync.dma_start(
                        out=f2x[:, b, :], in_=fj[b, nf].rearrange("p h w -> p (h w)")
                    )

                if mode == "dma":
                    for b in range(BPC):
                        o = outp.tile([P, G * G], f32, tag="o")
                        nc.vector.memset(o, 0.0)
                        nc.scalar.dma_start(
                            out=out_d[b, nf].rearrange("k g1 g2 -> k (g1 g2)"), in_=o
                        )
                    continue

                # f2 at the K selected grid points: gather the 4 bilinear
                # taps for both batches in one GPSIMD op
                g2 = work.tile([P, BPC, K, 4], f32, tag="g2")
                nc.gpsimd.ap_gather(
                    g2.rearrange("p b k t -> p (b k t)"),
                    f2x.rearrange("p b q -> p (b q)"),
                    gidx2_t[nf],
                    channels=P,
                    num_elems=BPC * H * W // 2,
                    d=2,
                    num_idxs=BPC * 2 * K,
                )

                for b in range(BPC):
                    # tap weights, then pre-sum the 4 taps -> single lhsT
                    gg = work.tile([P, K, 4], f32, tag="gg")
                    nc.vector.tensor_mul(
                        gg.rearrange("p k t -> p (k t)"),
                        g2[:, b].rearrange("p k t -> p (k t)"),
                        gw_t[nf],
                    )
                    ggv = gg.rearrange("p k (x two) -> p (k x) two", two=2)
                    h1 = work.tile([P, 2 * K], f32, tag="h1")
                    nc.vector.tensor_add(h1, ggv[:, :, 0], ggv[:, :, 1])
                    h1v = h1.rearrange("p (k two) -> p k two", two=2)
                    f2sel = work.tile([P, K], f32r, tag="f2sel")
                    nc.vector.tensor_add(f2sel, h1v[:, :, 0], h1v[:, :, 1])

                    # 4 weighted tap planes on the full 28x28 grid (DVE), the
                    # tap summation rides the PSUM accumulation
                    f1v = f1t[b].rearrange(
                        "p (h uu w tt) -> p h uu w tt", h=G, uu=2, w=G, tt=2
                    )
                    m = []
                    for u in range(2):
                        for t in range(2):
                            mt = work.tile([P, G * G], f32r, tag=f"m{u}{t}")
                            nc.vector.tensor_mul(
                                mt.rearrange("p (h w) -> p h w", w=G),
                                f1v[:, :, u, :, t],
                                w4_t[2 * u + t].rearrange("p (h w) -> p h w", w=G),
                            )
                            m.append(mt)

                    # corr[k, q] = sum_c f2sel[c,k] * sum_u m_u[c,q]
                    # single stationary operand, 4 accumulating matmuls per
                    # PSUM bank (784 cols split 392+392 to stay in-bank)
                    ps = pspool.tile([P, 2, 512], f32, tag="ps")
                    for half in range(2):
                        lo = half * QH
                        for u4 in range(4):
                            nc.tensor.matmul(
                                ps[:, half, :QH],
                                lhsT=f2sel,
                                rhs=m[u4][:, lo : lo + QH],
                                start=(u4 == 0),
                                stop=(u4 == 3),
                            )

                    # epilogue on ScalarE: r = 10*relu(corr); s = sum(exp(r/10));
                    # out = r * (1/s)
                    r = outp.tile([P, 2, QH], f32, tag="r")
                    nc.scalar.activation(r, ps[:, :, :QH], AF.Relu, scale=10.0)
                    rf = r.rearrange("p h q -> p (h q)")  # [P, 784] contiguous
                    e = outp.tile([P, G * G], f32, tag="e")
                    s = work.tile([P, 1], f32, tag="s")
                    nc.scalar.activation(e, rf, AF.Exp, scale=0.1, accum_out=s)
                    rec = work.tile([P, 1], f32, tag="rec")
                    nc.vector.reciprocal(rec, s)
                    o = outp.tile([P, G * G], f32, tag="o")
                    nc.scalar.mul(o, rf, rec)
                    # issue the store from ScalarE (mostly idle): keeps the
                    # SP/sync stream free to prefetch later pairs
                    nc.scalar.dma_start(
                        out=out_d[b, nf].rearrange("k g1 g2 -> k (g1 g2)"), in_=o
                    )
    return nc


def _get_bass():
    if "nc" not in _CACHE:
        nc = _build_bass()
        # run the Bacc passes (reg alloc, library-load insertion) before the
        # PJRT path serializes the module
        if not nc.is_finalized():
            nc.finalize()
        _CACHE["nc"] = nc
    return _CACHE["nc"]


def kernel(feature_i, feature_j, mask, optical_flow, knn_inds):
    from concourse import bass_utils

    nc = _get_bass()
    w4, gidx2, gwts = _host_consts(knn_inds)

    fi = np.ascontiguousarray(np.asarray(feature_i, dtype=np.float32))
    fj = np.ascontiguousarray(np.asarray(feature_j, dtype=np.float32))

    in_maps = []
    for core in range(NCORES):
        lo = core * BPC
        in_maps.append(
            {
                "fi": fi[lo : lo + BPC],
                "fj": fj[lo : lo + BPC],
                "w4": np.concatenate([w4.reshape(-1), np.ones(P, np.float32)])[None, :],
                "gidx2": gidx2,
                "gw": gwts.reshape(1, -1),
            }
        )

    res = bass_utils.run_bass_kernel_spmd(nc, in_maps, core_ids=list(range(NCORES)))
    out = np.concatenate([res.results[c]["out"] for c in range(NCORES)], axis=0)
    return out.astype(np.float32)
